# revision 13
# baseline (speedup 1.0000x reference)
"""Causal single-head attention (b=4, n=2048, d=1024) on 8 trn2 cores.

Sharding: 2 cores per batch element (pairs [0,1],[2,3],[4,5],[6,7]).
The V projection is split along d_out within a pair: rank r computes
only V[:, 512r:512r+512] and the halves are exchanged with a single
pairwise AllGather through DRAM bounce buffers. V is the LAST tensor
the attention needs (first AV matmul fires after all scores and
transposes), so the collective's ~60us firmware latency (a tiny warmup
AllGather is issued at t=0 to absorb its entry cost) hides entirely
under the K/Q projections and score computation. K^T is computed in
full on both cores of a pair (duplicating K costs +27us of PE but
removes a second collective whose serialized latency would stall the
score matmuls - measured net win).

Each batch's 16 query blocks (128 rows) are split by parity so every
core processes one q-block at each "capacity" in {2,4,...,16}
key-blocks; the instruction stream is identical on all cores (pure
SPMD) - only the data differs.

Compute is bf16 on the PE (tolerance 2e-2; measured ~6e-3): K/V/Q
projections, scores = Q^T.K per q-block, exp directly from PSUM on the
Scalar engine (no max subtraction - logits are O(5), safe in f32) with
fused row-sum accumulation, PE transpose of the exp'd weights, then
all AV matmuls emitted after all scores so V is only needed late.
1/sqrt(d) = 2^-5 is folded into Q^T; 1/rowsum is folded into the
PSUM->SBUF copyback of AV.
"""

import numpy as np

P = 128
B, N, D = 4, 2048, 1024
NCORES = 8
CAPS = (16, 14, 12, 10, 8, 6, 4, 2)  # key-block capacity per slot
SUMCAPS = sum(CAPS)  # 72 key-block visits per core
NEG = -1.0e30
PAIRS = [[0, 1], [2, 3], [4, 5], [6, 7]]
EH = D // 2  # 512: e-columns of V computed locally per core

MM_DT = "bf16"  # informational; test.py prints it

_prog_cache = {}


def _split_multi_waits(nc, max_waits=1):
    """walrus in this container rejects more than one sem wait per
    instruction ("Too many sync wait commands"). After Tile scheduling,
    hoist extra waits onto same-engine nops inserted just before the
    instruction (same blocking semantics: engine queues are in-order)."""
    from concourse import mybir

    n = 0
    for fn in nc.m.functions:
        for bb in fn.blocks:
            out = []
            for ins in bb.instructions:
                si = ins.sync_info
                waits = list(si.on_wait) if si and si.on_wait else []
                if len(waits) > max_waits:
                    extra = waits[:-max_waits]
                    si.on_wait = waits[-max_waits:]
                    for j in range(0, len(extra), max_waits):
                        nop = mybir.InstNoOp(
                            name=f"waitsplit_{n}", ins=[], outs=[],
                            engine=ins.engine)
                        n += 1
                        nop.sync_info = mybir.SyncInfo(
                            on_wait=extra[j:j + max_waits], on_update=[])
                        out.append(nop)
                out.append(ins)
            bb.instructions[:] = out


def _build_program():
    import concourse.bass as bass
    import concourse.tile as tile
    from concourse import mybir
    from concourse.masks import make_identity

    f32 = mybir.dt.float32
    bf16 = mybir.dt.bfloat16
    DC = D // P  # 8 contraction chunks

    nc = bass.Bass("TRN2", target_bir_lowering=False, debug=False,
                   num_devices=NCORES, dynamic_dma_scratch_size=2048)

    xT_d = nc.dram_tensor("xT", [D, N], bf16, kind="ExternalInput").ap()
    xqT_d = nc.dram_tensor("xqT", [D, 8 * P], bf16, kind="ExternalInput").ap()
    wq_d = nc.dram_tensor("wq", [D, D], bf16, kind="ExternalInput").ap()
    wk_d = nc.dram_tensor("wk", [D, D], bf16, kind="ExternalInput").ap()
    wvh_d = nc.dram_tensor("wvh", [D, EH], bf16, kind="ExternalInput").ap()
    mask_d = nc.dram_tensor("mask", [P, 2 * P], f32, kind="ExternalInput").ap()
    out_d = nc.dram_tensor("out", [8 * P, D], f32, kind="ExternalOutput").ap()

    # AllGather bounce buffers (pairwise, rank-major gather along dim 0)
    warm_i = nc.dram_tensor("warm_i", [P, 16], bf16, kind="Internal").ap()
    warm_o = nc.dram_tensor("warm_o", [2, P, 16], bf16, kind="Internal").ap()
    vag_i = nc.dram_tensor("vag_i", [N // P, P, EH], bf16, kind="Internal").ap()
    vag_o = nc.dram_tensor("vag_o", [2, N // P, P, EH], bf16,
                           kind="Internal").ap()

    xT_r = xT_d.rearrange("(dc p) k -> p dc k", p=P)
    xqT_r = xqT_d.rearrange("(dc p) q -> p dc q", p=P)
    wq_r = wq_d.rearrange("(dc p) e -> p dc e", p=P)
    wk_r = wk_d.rearrange("(dc p) e -> p dc e", p=P)
    wvh_r = wvh_d.rearrange("(dc p) e -> p dc e", p=P)

    offs = [128 * sum(CAPS[:s]) for s in range(len(CAPS))]  # sc/wt offsets

    def ag(ins_ap, outs_ap):
        nc.gpsimd.collective_compute(
            "AllGather", mybir.AluOpType.bypass, replica_groups=PAIRS,
            ins=[ins_ap], outs=[outs_ap])

    with tile.TileContext(nc) as tc:
        import contextlib
        with contextlib.ExitStack() as ctx:
            cpool = ctx.enter_context(tc.tile_pool(name="cpool", bufs=1))
            qtp = ctx.enter_context(tc.tile_pool(name="qtp", bufs=1))
            ktp = ctx.enter_context(tc.tile_pool(name="ktp", bufs=1))
            vp = ctx.enter_context(tc.tile_pool(name="vp", bufs=1))
            scp = ctx.enter_context(tc.tile_pool(name="scp", bufs=1))
            wtp = ctx.enter_context(tc.tile_pool(name="wtp", bufs=1))
            stp = ctx.enter_context(tc.tile_pool(name="stp", bufs=1))

            # ---- warmup AllGather: absorbs the ~35us firmware wakeup so
            # the real V AllGather starts promptly once its input lands ----
            warm = cpool.tile([P, 16], bf16, name="warm")
            nc.vector.memset(warm[:], 0.0)
            nc.sync.dma_start(warm_i, warm[:])
            ag(warm_i, warm_o)

            QT = qtp.tile([P, DC, 8 * P], bf16, name="QT")
            KT = ktp.tile([P, DC, N], bf16, name="KT")
            V = vp.tile([P, N // P, D], bf16, name="V")
            SC = scp.tile([P, SUMCAPS * P], bf16, name="SC")  # exp'd scores
            WT = wtp.tile([P, SUMCAPS * P], bf16, name="WT")  # transposed
            ST = stp.tile([P, len(CAPS), 6], f32, name="ST")  # rowsum stats

            # ---- projections ----
            with tc.tile_pool(name="wpool", bufs=1) as wpool, \
                 tc.tile_pool(name="xpool", bufs=2) as xpool, \
                 tc.tile_pool(name="cst", bufs=3) as cst, \
                 tc.tile_pool(name="ppj", bufs=4, space="PSUM") as ppj:

                # first x chunk + V-half weights, interleaved halves so the
                # PE can start after ~1MB of DMA; then prefetch everything
                # else the projections will need (nothing below has waits,
                # so the sync queue streams it all back-to-back)
                xs0 = xpool.tile([P, DC, 512], bf16, tag="xs", name="xs_v0")
                wvh = wpool.tile([P, DC, EH], bf16, name="wvh")
                for h4 in range(2):
                    sl = slice(4 * h4, 4 * h4 + 4)
                    nc.sync.dma_start(xs0[:, sl, :], xT_r[:, sl, 0:512])
                    nc.sync.dma_start(wvh[:, sl, :], wvh_r[:, sl, :])
                wk = wpool.tile([P, DC, D], bf16, name="wk")
                wq = wpool.tile([P, DC, D], bf16, name="wq")
                nc.sync.dma_start(wk[:], wk_r)
                nc.sync.dma_start(wq[:], wq_r)
                mask_sb = cpool.tile([P, 2 * P], f32, name="mask_sb")
                nc.sync.dma_start(mask_sb[:], mask_d)
                ident_f = cpool.tile([P, P], f32, name="ident_f")
                make_identity(nc, ident_f)
                ident = cpool.tile([P, P], bf16, name="ident")
                nc.vector.tensor_copy(ident[:], ident_f[:])

                # V e-half: V[k, 512r + j] for j in 0..511, staged to the
                # AllGather bounce as soon as each 128-row block is cast
                for c in range(4):
                    if c == 0:
                        xs = xs0
                    else:
                        xs = xpool.tile([P, DC, 512], bf16, tag="xs",
                                        name="xs_v")
                        nc.sync.dma_start(
                            xs[:], xT_r[:, :, c * 512:(c + 1) * 512])
                    for kb in range(4):
                        ps = ppj.tile([P, 512], f32, tag="pj", name="ps_v")
                        for dc in range(DC):
                            nc.tensor.matmul(
                                ps,
                                xs[:, dc, kb * P:(kb + 1) * P],
                                wvh[:, dc, :],
                                start=(dc == 0), stop=(dc == DC - 1))
                        vst = cst.tile([P, 512], bf16, tag="cst", name="vst")
                        nc.vector.tensor_copy(vst[:], ps)
                        # scalar (ACT) HWDGE queue: keeps these cast-gated
                        # stores from head-blocking the sync DMA queue
                        nc.scalar.dma_start(vag_i[4 * c + kb], vst[:])
                ag(vag_i, vag_o)

                # K^T in full (duplicated within the pair)
                for c in range(4):
                    xs = xpool.tile([P, DC, 512], bf16, tag="xs", name="xs_k")
                    nc.sync.dma_start(xs[:], xT_r[:, :, c * 512:(c + 1) * 512])
                    for ec in range(DC):
                        ps = ppj.tile([P, 512], f32, tag="pj", name="ps_k")
                        for dc in range(DC):
                            nc.tensor.matmul(
                                ps,
                                wk[:, dc, ec * P:(ec + 1) * P],
                                xs[:, dc, :],
                                start=(dc == 0), stop=(dc == DC - 1))
                        nc.vector.tensor_copy(
                            KT[:, ec, c * 512:(c + 1) * 512], ps)

                # V readback from the gathered halves (scalar engine queue
                # so other sync DMAs don't queue behind the AG wait);
                # split per half so AV h=0 can start before h=1 lands
                nc.scalar.dma_start(
                    V[:, :, 0:EH], vag_o[0].rearrange("kc p e -> p kc e"))
                nc.scalar.dma_start(
                    V[:, :, EH:D], vag_o[1].rearrange("kc p e -> p kc e"))

                # Q^T full e for own q rows, scaled by 1/32
                for qc in range(2):
                    xs = xpool.tile([P, DC, 512], bf16, tag="xs", name="xs_q")
                    nc.sync.dma_start(
                        xs[:], xqT_r[:, :, qc * 512:(qc + 1) * 512])
                    for ec in range(DC):
                        ps = ppj.tile([P, 512], f32, tag="pj", name="ps_q")
                        for dc in range(DC):
                            nc.tensor.matmul(
                                ps,
                                wq[:, dc, ec * P:(ec + 1) * P],
                                xs[:, dc, :],
                                start=(dc == 0), stop=(dc == DC - 1))
                        nc.vector.tensor_scalar_mul(
                            QT[:, ec, qc * 512:(qc + 1) * 512], ps, 1.0 / 32.0)

            # ---- attention ----
            with tc.tile_pool(name="psc", bufs=2, space="PSUM") as psc, \
                 tc.tile_pool(name="ptr", bufs=2, space="PSUM") as ptr, \
                 tc.tile_pool(name="pav", bufs=4, space="PSUM") as pav, \
                 tc.tile_pool(name="obp", bufs=2) as obp:

                # scores + exp + transpose, slot by slot (V-independent)
                for s in range(len(CAPS)):
                    L = CAPS[s] * P
                    off = offs[s]
                    widths = [512] * (L // 512) + ([256] if L % 512 else [])
                    koff = 0
                    for ci, w in enumerate(widths):
                        ps = psc.tile([P, 512], f32, tag="psc", name=f"sc{s}")
                        for ec in range(DC):
                            nc.tensor.matmul(
                                ps[:, :w],
                                QT[:, ec, s * P:(s + 1) * P],
                                KT[:, ec, koff:koff + w],
                                start=(ec == 0), stop=(ec == DC - 1))
                        koff += w
                        if koff == L:  # apply causal mask to last 256 cols
                            nc.vector.tensor_add(
                                ps[:, w - 256:w], ps[:, w - 256:w], mask_sb[:])
                        nc.scalar.activation(
                            SC[:, off + koff - w:off + koff], ps[:, :w],
                            mybir.ActivationFunctionType.Exp,
                            accum_out=ST[:, s, ci:ci + 1])
                    # rowsum = sum of chunk accumulators; reciprocal
                    nc.vector.tensor_reduce(
                        ST[:, s, 4:5], ST[:, s, 0:len(widths)],
                        axis=mybir.AxisListType.X, op=mybir.AluOpType.add)
                    nc.vector.reciprocal(ST[:, s, 5:6], ST[:, s, 4:5])
                    for j in range(CAPS[s]):
                        pt = ptr.tile([P, P], bf16, tag="ptr", name=f"pt{s}")
                        nc.tensor.transpose(
                            pt, SC[:, off + j * P:off + (j + 1) * P], ident)
                        nc.vector.tensor_copy(
                            WT[:, off + j * P:off + (j + 1) * P], pt)

                # AV, slot by slot (first consumer of V); h-outer so the
                # h=0 matmuls only need the first V readback DMA
                for s in range(len(CAPS)):
                    off = offs[s]
                    avs = [pav.tile([P, 512], f32, tag="pav",
                                    name=f"av{s}_{h}") for h in range(2)]
                    for h in range(2):
                        for j in range(CAPS[s]):
                            nc.tensor.matmul(
                                avs[h],
                                WT[:, off + j * P:off + (j + 1) * P],
                                V[:, j, h * 512:(h + 1) * 512],
                                start=(j == 0), stop=(j == CAPS[s] - 1))
                    ob = obp.tile([P, D], f32, tag="ob", name=f"ob{s}")
                    for h in range(2):
                        nc.vector.tensor_scalar_mul(
                            ob[:, h * 512:(h + 1) * 512], avs[h], ST[:, s, 5:6])
                    nc.sync.dma_start(out_d[s * P:(s + 1) * P, :], ob)

    _split_multi_waits(nc)
    return nc


def _host_prep(x, Wq, Wk, Wv):
    """Build per-core input maps."""
    import ml_dtypes
    bf16 = ml_dtypes.bfloat16

    x = np.ascontiguousarray(x, dtype=np.float32)
    tri = np.where(
        np.arange(P)[None, :] <= np.arange(P)[:, None], 0.0, NEG
    ).astype(np.float32)
    mask_even = np.concatenate(  # parity 0: diag block then fully-masked block
        [tri, np.full((P, P), NEG, np.float32)], axis=1)
    mask_odd = np.concatenate(  # parity 1: fully-visible block then diag block
        [np.zeros((P, P), np.float32), tri], axis=1)

    xb = [np.ascontiguousarray(x[bi].T).astype(bf16) for bi in range(B)]
    wq_b = np.ascontiguousarray(Wq, dtype=np.float32).astype(bf16)
    wk_b = np.ascontiguousarray(Wk, dtype=np.float32).astype(bf16)
    wv_b = np.ascontiguousarray(Wv, dtype=np.float32).astype(bf16)

    in_maps = []
    for c in range(NCORES):
        bi, r = c // 2, c % 2
        rbs = [s - 2 + r for s in CAPS]
        xq = np.concatenate([x[bi, rb * P:(rb + 1) * P, :] for rb in rbs],
                            axis=0)
        in_maps.append({
            "xT": xb[bi],
            "xqT": np.ascontiguousarray(xq.T).astype(bf16),
            "wq": wq_b,
            "wk": wk_b,
            "wvh": np.ascontiguousarray(wv_b[:, r * EH:(r + 1) * EH]),
            "mask": mask_odd if r else mask_even,
        })
    return in_maps


def _host_gather(results):
    out = np.empty((B, N, D), dtype=np.float32)
    for c in range(NCORES):
        bi, r = c // 2, c % 2
        res = results[c]["out"]
        for k, s in enumerate(CAPS):
            rb = s - 2 + r
            out[bi, rb * P:(rb + 1) * P, :] = res[k * P:(k + 1) * P, :]
    return out


def kernel(x, Wq, Wk, Wv, _trace=False, _trace_kwargs=None):
    from concourse.bass_utils import run_bass_kernel_spmd

    if "prog" not in _prog_cache:
        _prog_cache["prog"] = _build_program()
    nc = _prog_cache["prog"]

    in_maps = _host_prep(x, Wq, Wk, Wv)
    kw = dict(_trace_kwargs or {})
    res = run_bass_kernel_spmd(nc, in_maps, list(range(NCORES)),
                               trace=_trace, **kw)
    out = _host_gather(res.results)
    if _trace:
        return out, res
    return out


# revision 15
# speedup vs baseline: 1.0249x; 1.0249x over previous
"""Causal single-head attention (b=4, n=2048, d=1024) on 8 trn2 cores.

Sharding: 2 cores per batch element (pairs [0,1],[2,3],[4,5],[6,7]).
The V projection is split along d_out within a pair: rank r computes
only V[:, 512r:512r+512] and the halves are exchanged with a single
pairwise AllGather through DRAM bounce buffers. V is the LAST tensor
the attention needs (first AV matmul fires after all scores and
transposes), so the collective's ~60us firmware latency (a tiny warmup
AllGather is issued at t=0 to absorb its entry cost) hides entirely
under the K/Q projections and score computation. K^T is computed in
full on both cores of a pair (duplicating K costs +27us of PE but
removes a second collective whose serialized latency would stall the
score matmuls - measured net win).

Each batch's 16 query blocks (128 rows) are split by parity so every
core processes one q-block at each "capacity" in {2,4,...,16}
key-blocks; the instruction stream is identical on all cores (pure
SPMD) - only the data differs.

Compute is bf16 on the PE (tolerance 2e-2; measured ~6e-3): K/V/Q
projections, scores = Q^T.K per q-block, exp directly from PSUM on the
Scalar engine (no max subtraction - logits are O(5), safe in f32) with
fused row-sum accumulation, PE transpose of the exp'd weights, then
all AV matmuls emitted after all scores so V is only needed late.
1/sqrt(d) = 2^-5 is folded into Q^T; 1/rowsum is folded into the
PSUM->SBUF copyback of AV.
"""

import numpy as np

P = 128
B, N, D = 4, 2048, 1024
NCORES = 8
CAPS = (16, 14, 12, 10, 8, 6, 4, 2)  # key-block capacity per slot
SUMCAPS = sum(CAPS)  # 72 key-block visits per core
NEG = -1.0e30
PAIRS = [[0, 1], [2, 3], [4, 5], [6, 7]]
EH = D // 2  # 512: e-columns of V computed locally per core

MM_DT = "bf16"  # informational; test.py prints it

_prog_cache = {}


def _split_multi_waits(nc, max_waits=1):
    """walrus in this container rejects more than one sem wait per
    instruction ("Too many sync wait commands"). After Tile scheduling,
    hoist extra waits onto same-engine nops inserted just before the
    instruction (same blocking semantics: engine queues are in-order)."""
    from concourse import mybir

    n = 0
    for fn in nc.m.functions:
        for bb in fn.blocks:
            out = []
            for ins in bb.instructions:
                si = ins.sync_info
                waits = list(si.on_wait) if si and si.on_wait else []
                if len(waits) > max_waits:
                    extra = waits[:-max_waits]
                    si.on_wait = waits[-max_waits:]
                    for j in range(0, len(extra), max_waits):
                        nop = mybir.InstNoOp(
                            name=f"waitsplit_{n}", ins=[], outs=[],
                            engine=ins.engine)
                        n += 1
                        nop.sync_info = mybir.SyncInfo(
                            on_wait=extra[j:j + max_waits], on_update=[])
                        out.append(nop)
                out.append(ins)
            bb.instructions[:] = out


def _build_program():
    import concourse.bass as bass
    import concourse.tile as tile
    from concourse import mybir
    from concourse.masks import make_identity

    f32 = mybir.dt.float32
    bf16 = mybir.dt.bfloat16
    DC = D // P  # 8 contraction chunks

    nc = bass.Bass("TRN2", target_bir_lowering=False, debug=False,
                   num_devices=NCORES, dynamic_dma_scratch_size=2048)

    xT_d = nc.dram_tensor("xT", [D, N], bf16, kind="ExternalInput").ap()
    xqT_d = nc.dram_tensor("xqT", [D, 8 * P], bf16, kind="ExternalInput").ap()
    wq_d = nc.dram_tensor("wq", [D, D], bf16, kind="ExternalInput").ap()
    wk_d = nc.dram_tensor("wk", [D, D], bf16, kind="ExternalInput").ap()
    wvh_d = nc.dram_tensor("wvh", [D, EH], bf16, kind="ExternalInput").ap()
    mask_d = nc.dram_tensor("mask", [P, 2 * P], f32, kind="ExternalInput").ap()
    out_d = nc.dram_tensor("out", [8 * P, D], f32, kind="ExternalOutput").ap()

    # AllGather bounce buffers (pairwise, rank-major gather along dim 0)
    warm_i = nc.dram_tensor("warm_i", [P, 16], bf16, kind="Internal").ap()
    warm_o = nc.dram_tensor("warm_o", [2, P, 16], bf16, kind="Internal").ap()
    vag_i = nc.dram_tensor("vag_i", [N // P, P, EH], bf16, kind="Internal").ap()
    vag_o = nc.dram_tensor("vag_o", [2, N // P, P, EH], bf16,
                           kind="Internal").ap()

    xT_r = xT_d.rearrange("(dc p) k -> p dc k", p=P)
    xqT_r = xqT_d.rearrange("(dc p) q -> p dc q", p=P)
    wq_r = wq_d.rearrange("(dc p) e -> p dc e", p=P)
    wk_r = wk_d.rearrange("(dc p) e -> p dc e", p=P)
    wvh_r = wvh_d.rearrange("(dc p) e -> p dc e", p=P)

    offs = [128 * sum(CAPS[:s]) for s in range(len(CAPS))]  # sc/wt offsets

    def ag(ins_ap, outs_ap):
        nc.gpsimd.collective_compute(
            "AllGather", mybir.AluOpType.bypass, replica_groups=PAIRS,
            ins=[ins_ap], outs=[outs_ap])

    with tile.TileContext(nc) as tc:
        import contextlib
        with contextlib.ExitStack() as ctx:
            cpool = ctx.enter_context(tc.tile_pool(name="cpool", bufs=1))
            qtp = ctx.enter_context(tc.tile_pool(name="qtp", bufs=1))
            ktp = ctx.enter_context(tc.tile_pool(name="ktp", bufs=1))
            vp = ctx.enter_context(tc.tile_pool(name="vp", bufs=1))
            scp = ctx.enter_context(tc.tile_pool(name="scp", bufs=1))
            wtp = ctx.enter_context(tc.tile_pool(name="wtp", bufs=1))
            stp = ctx.enter_context(tc.tile_pool(name="stp", bufs=1))

            # ---- warmup AllGather: absorbs the ~35us firmware wakeup so
            # the real V AllGather starts promptly once its input lands ----
            warm = cpool.tile([P, 16], bf16, name="warm")
            nc.vector.memset(warm[:], 0.0)
            nc.sync.dma_start(warm_i, warm[:])
            ag(warm_i, warm_o)

            QT = qtp.tile([P, DC, 8 * P], bf16, name="QT")
            KT = ktp.tile([P, DC, N], bf16, name="KT")
            V = vp.tile([P, N // P, D], bf16, name="V")

            # ---- projections ----
            with tc.tile_pool(name="wpool", bufs=1) as wpool, \
                 tc.tile_pool(name="xfp", bufs=1) as xfp, \
                 tc.tile_pool(name="xpool", bufs=2) as xpool, \
                 tc.tile_pool(name="cst", bufs=3) as cst, \
                 tc.tile_pool(name="ppj", bufs=4, space="PSUM") as ppj:

                # x^T fully resident (reused by the V then K projections).
                # First 512-col chunk is interleaved with the V weights in
                # dc-quarters so the PE can start after ~0.5MB of DMA; the
                # rest streams behind it, weights after x (nothing below
                # has waits, so the sync queue issues back-to-back).
                xf = xfp.tile([P, DC, N], bf16, name="xf")
                wvh = wpool.tile([P, DC, EH], bf16, name="wvh")
                for q4 in range(4):
                    sl = slice(2 * q4, 2 * q4 + 2)
                    nc.sync.dma_start(xf[:, sl, 0:512], xT_r[:, sl, 0:512])
                    nc.sync.dma_start(wvh[:, sl, :], wvh_r[:, sl, :])
                for c in range(1, 4):
                    nc.sync.dma_start(xf[:, :, c * 512:(c + 1) * 512],
                                      xT_r[:, :, c * 512:(c + 1) * 512])
                wk = wpool.tile([P, DC, D], bf16, name="wk")
                wq = wpool.tile([P, DC, D], bf16, name="wq")
                nc.sync.dma_start(wk[:], wk_r)
                nc.sync.dma_start(wq[:], wq_r)
                mask_sb = cpool.tile([P, 2 * P], f32, name="mask_sb")
                nc.sync.dma_start(mask_sb[:], mask_d)
                ident_f = cpool.tile([P, P], f32, name="ident_f")
                make_identity(nc, ident_f)
                ident = cpool.tile([P, P], bf16, name="ident")
                nc.vector.tensor_copy(ident[:], ident_f[:])

                # V e-half: V[k, 512r + j] for j in 0..511, staged to the
                # AllGather bounce as soon as each 128-row block is cast
                for c in range(4):
                    for kb in range(4):
                        ps = ppj.tile([P, 512], f32, tag="pj", name="ps_v")
                        for dc in range(DC):
                            nc.tensor.matmul(
                                ps,
                                xf[:, dc, (4 * c + kb) * P:(4 * c + kb + 1) * P],
                                wvh[:, dc, :],
                                start=(dc == 0), stop=(dc == DC - 1))
                        vst = cst.tile([P, 512], bf16, tag="cst", name="vst")
                        nc.vector.tensor_copy(vst[:], ps)
                        # scalar (ACT) HWDGE queue: keeps these cast-gated
                        # stores from head-blocking the sync DMA queue
                        nc.scalar.dma_start(vag_i[4 * c + kb], vst[:])
                ag(vag_i, vag_o)

                # K^T in full (duplicated within the pair)
                for c in range(4):
                    for ec in range(DC):
                        ps = ppj.tile([P, 512], f32, tag="pj", name="ps_k")
                        for dc in range(DC):
                            nc.tensor.matmul(
                                ps,
                                wk[:, dc, ec * P:(ec + 1) * P],
                                xf[:, dc, c * 512:(c + 1) * 512],
                                start=(dc == 0), stop=(dc == DC - 1))
                        nc.vector.tensor_copy(
                            KT[:, ec, c * 512:(c + 1) * 512], ps)

                # V readback from the gathered halves (scalar engine queue
                # so other sync DMAs don't queue behind the AG wait);
                # split per half so AV h=0 can start before h=1 lands
                nc.scalar.dma_start(
                    V[:, :, 0:EH], vag_o[0].rearrange("kc p e -> p kc e"))
                nc.scalar.dma_start(
                    V[:, :, EH:D], vag_o[1].rearrange("kc p e -> p kc e"))

                # Q^T full e for own q rows, scaled by 1/32
                for qc in range(2):
                    xs = xpool.tile([P, DC, 512], bf16, tag="xs", name="xs_q")
                    nc.sync.dma_start(
                        xs[:], xqT_r[:, :, qc * 512:(qc + 1) * 512])
                    for ec in range(DC):
                        ps = ppj.tile([P, 512], f32, tag="pj", name="ps_q")
                        for dc in range(DC):
                            nc.tensor.matmul(
                                ps,
                                wq[:, dc, ec * P:(ec + 1) * P],
                                xs[:, dc, :],
                                start=(dc == 0), stop=(dc == DC - 1))
                        nc.vector.tensor_scalar_mul(
                            QT[:, ec, qc * 512:(qc + 1) * 512], ps, 1.0 / 32.0)

            # ---- attention ----
            with tc.tile_pool(name="psc", bufs=2, space="PSUM") as psc, \
                 tc.tile_pool(name="ptr", bufs=2, space="PSUM") as ptr, \
                 tc.tile_pool(name="pav", bufs=4, space="PSUM") as pav, \
                 tc.tile_pool(name="obp", bufs=2) as obp:

                SC = scp.tile([P, SUMCAPS * P], bf16, name="SC")  # exp'd
                WT = wtp.tile([P, SUMCAPS * P], bf16, name="WT")  # transposed
                ST = stp.tile([P, len(CAPS), 6], f32, name="ST")  # rowsums

                # scores + exp + transpose, slot by slot (V-independent)
                for s in range(len(CAPS)):
                    L = CAPS[s] * P
                    off = offs[s]
                    widths = [512] * (L // 512) + ([256] if L % 512 else [])
                    koff = 0
                    for ci, w in enumerate(widths):
                        ps = psc.tile([P, 512], f32, tag="psc", name=f"sc{s}")
                        for ec in range(DC):
                            nc.tensor.matmul(
                                ps[:, :w],
                                QT[:, ec, s * P:(s + 1) * P],
                                KT[:, ec, koff:koff + w],
                                start=(ec == 0), stop=(ec == DC - 1))
                        koff += w
                        if koff == L:  # apply causal mask to last 256 cols
                            nc.vector.tensor_add(
                                ps[:, w - 256:w], ps[:, w - 256:w], mask_sb[:])
                        nc.scalar.activation(
                            SC[:, off + koff - w:off + koff], ps[:, :w],
                            mybir.ActivationFunctionType.Exp,
                            accum_out=ST[:, s, ci:ci + 1])
                    # rowsum = sum of chunk accumulators; reciprocal
                    nc.vector.tensor_reduce(
                        ST[:, s, 4:5], ST[:, s, 0:len(widths)],
                        axis=mybir.AxisListType.X, op=mybir.AluOpType.add)
                    nc.vector.reciprocal(ST[:, s, 5:6], ST[:, s, 4:5])
                    for j in range(CAPS[s]):
                        pt = ptr.tile([P, P], bf16, tag="ptr", name=f"pt{s}")
                        nc.tensor.transpose(
                            pt, SC[:, off + j * P:off + (j + 1) * P], ident)
                        nc.vector.tensor_copy(
                            WT[:, off + j * P:off + (j + 1) * P], pt)

                # AV, slot by slot (first consumer of V); h-outer so the
                # h=0 matmuls only need the first V readback DMA
                for s in range(len(CAPS)):
                    off = offs[s]
                    avs = [pav.tile([P, 512], f32, tag="pav",
                                    name=f"av{s}_{h}") for h in range(2)]
                    for h in range(2):
                        for j in range(CAPS[s]):
                            nc.tensor.matmul(
                                avs[h],
                                WT[:, off + j * P:off + (j + 1) * P],
                                V[:, j, h * 512:(h + 1) * 512],
                                start=(j == 0), stop=(j == CAPS[s] - 1))
                    ob = obp.tile([P, D], f32, tag="ob", name=f"ob{s}")
                    for h in range(2):
                        nc.vector.tensor_scalar_mul(
                            ob[:, h * 512:(h + 1) * 512], avs[h], ST[:, s, 5:6])
                    nc.sync.dma_start(out_d[s * P:(s + 1) * P, :], ob)

    _split_multi_waits(nc)
    return nc


def _host_prep(x, Wq, Wk, Wv):
    """Build per-core input maps."""
    import ml_dtypes
    bf16 = ml_dtypes.bfloat16

    x = np.ascontiguousarray(x, dtype=np.float32)
    tri = np.where(
        np.arange(P)[None, :] <= np.arange(P)[:, None], 0.0, NEG
    ).astype(np.float32)
    mask_even = np.concatenate(  # parity 0: diag block then fully-masked block
        [tri, np.full((P, P), NEG, np.float32)], axis=1)
    mask_odd = np.concatenate(  # parity 1: fully-visible block then diag block
        [np.zeros((P, P), np.float32), tri], axis=1)

    xb = [np.ascontiguousarray(x[bi].T).astype(bf16) for bi in range(B)]
    wq_b = np.ascontiguousarray(Wq, dtype=np.float32).astype(bf16)
    wk_b = np.ascontiguousarray(Wk, dtype=np.float32).astype(bf16)
    wv_b = np.ascontiguousarray(Wv, dtype=np.float32).astype(bf16)

    in_maps = []
    for c in range(NCORES):
        bi, r = c // 2, c % 2
        rbs = [s - 2 + r for s in CAPS]
        xq = np.concatenate([x[bi, rb * P:(rb + 1) * P, :] for rb in rbs],
                            axis=0)
        in_maps.append({
            "xT": xb[bi],
            "xqT": np.ascontiguousarray(xq.T).astype(bf16),
            "wq": wq_b,
            "wk": wk_b,
            "wvh": np.ascontiguousarray(wv_b[:, r * EH:(r + 1) * EH]),
            "mask": mask_odd if r else mask_even,
        })
    return in_maps


def _host_gather(results):
    out = np.empty((B, N, D), dtype=np.float32)
    for c in range(NCORES):
        bi, r = c // 2, c % 2
        res = results[c]["out"]
        for k, s in enumerate(CAPS):
            rb = s - 2 + r
            out[bi, rb * P:(rb + 1) * P, :] = res[k * P:(k + 1) * P, :]
    return out


def kernel(x, Wq, Wk, Wv, _trace=False, _trace_kwargs=None):
    from concourse.bass_utils import run_bass_kernel_spmd

    if "prog" not in _prog_cache:
        _prog_cache["prog"] = _build_program()
    nc = _prog_cache["prog"]

    in_maps = _host_prep(x, Wq, Wk, Wv)
    kw = dict(_trace_kwargs or {})
    res = run_bass_kernel_spmd(nc, in_maps, list(range(NCORES)),
                               trace=_trace, **kw)
    out = _host_gather(res.results)
    if _trace:
        return out, res
    return out


# revision 16
# speedup vs baseline: 1.0950x; 1.0683x over previous
"""Causal single-head attention (b=4, n=2048, d=1024) on 8 trn2 cores.

Sharding: 2 cores per batch element (pairs [0,1],[2,3],[4,5],[6,7]).
The V projection is split along d_out within a pair: rank r computes
only V[:, 512r:512r+512] and the halves are exchanged with a single
pairwise AllGather through DRAM bounce buffers. V is the LAST tensor
the attention needs (first AV matmul fires after all scores and
transposes), so the collective's ~60us firmware latency (a tiny warmup
AllGather is issued at t=0 to absorb its entry cost) hides entirely
under the K/Q projections and score computation. K^T is computed in
full on both cores of a pair (duplicating K costs +27us of PE but
removes a second collective whose serialized latency would stall the
score matmuls - measured net win).

Each batch's 16 query blocks (128 rows) are split by parity so every
core processes one q-block at each "capacity" in {2,4,...,16}
key-blocks; the instruction stream is identical on all cores (pure
SPMD) - only the data differs.

Compute is bf16 on the PE (tolerance 2e-2; measured ~6e-3): K/V/Q
projections, scores = Q^T.K per q-block, exp directly from PSUM on the
Scalar engine (no max subtraction - logits are O(5), safe in f32) with
fused row-sum accumulation, PE transpose of the exp'd weights, then
all AV matmuls emitted after all scores so V is only needed late.
1/sqrt(d) = 2^-5 is folded into Q^T; 1/rowsum is folded into the
PSUM->SBUF copyback of AV.
"""

import numpy as np

P = 128
B, N, D = 4, 2048, 1024
NCORES = 8
CAPS = (16, 14, 12, 10, 8, 6, 4, 2)  # key-block capacity per slot
SUMCAPS = sum(CAPS)  # 72 key-block visits per core
NEG = -1.0e30
PAIRS = [[0, 1], [2, 3], [4, 5], [6, 7]]
EH = D // 2  # 512: e-columns of V computed locally per core

MM_DT = "bf16"  # informational; test.py prints it

_prog_cache = {}


def _split_multi_waits(nc, max_waits=1):
    """walrus in this container rejects more than one sem wait per
    instruction ("Too many sync wait commands"). After Tile scheduling,
    hoist extra waits onto same-engine nops inserted just before the
    instruction (same blocking semantics: engine queues are in-order)."""
    from concourse import mybir

    n = 0
    for fn in nc.m.functions:
        for bb in fn.blocks:
            out = []
            for ins in bb.instructions:
                si = ins.sync_info
                waits = list(si.on_wait) if si and si.on_wait else []
                if len(waits) > max_waits:
                    extra = waits[:-max_waits]
                    si.on_wait = waits[-max_waits:]
                    for j in range(0, len(extra), max_waits):
                        nop = mybir.InstNoOp(
                            name=f"waitsplit_{n}", ins=[], outs=[],
                            engine=ins.engine)
                        n += 1
                        nop.sync_info = mybir.SyncInfo(
                            on_wait=extra[j:j + max_waits], on_update=[])
                        out.append(nop)
                out.append(ins)
            bb.instructions[:] = out


def _build_program():
    import concourse.bass as bass
    import concourse.tile as tile
    from concourse import mybir
    from concourse.masks import make_identity

    f32 = mybir.dt.float32
    bf16 = mybir.dt.bfloat16
    DC = D // P  # 8 contraction chunks

    nc = bass.Bass("TRN2", target_bir_lowering=False, debug=False,
                   num_devices=NCORES, dynamic_dma_scratch_size=2048)

    xT_d = nc.dram_tensor("xT", [D, N], bf16, kind="ExternalInput").ap()
    xqT_d = nc.dram_tensor("xqT", [D, 8 * P], bf16, kind="ExternalInput").ap()
    wq_d = nc.dram_tensor("wq", [D, D], bf16, kind="ExternalInput").ap()
    wk_d = nc.dram_tensor("wk", [D, D], bf16, kind="ExternalInput").ap()
    wvh_d = nc.dram_tensor("wvh", [D, EH], bf16, kind="ExternalInput").ap()
    mask_d = nc.dram_tensor("mask", [P, 2 * P], f32, kind="ExternalInput").ap()
    out_d = nc.dram_tensor("out", [8 * P, D], f32, kind="ExternalOutput").ap()

    # AllGather bounce buffers (pairwise, rank-major gather along dim 0)
    warm_i = nc.dram_tensor("warm_i", [P, 16], bf16, kind="Internal").ap()
    warm_o = nc.dram_tensor("warm_o", [2, P, 16], bf16, kind="Internal").ap()
    vag_i = nc.dram_tensor("vag_i", [N // P, P, EH], bf16, kind="Internal").ap()
    vag_o = nc.dram_tensor("vag_o", [2, N // P, P, EH], bf16,
                           kind="Internal").ap()

    xT_r = xT_d.rearrange("(dc p) k -> p dc k", p=P)
    xqT_r = xqT_d.rearrange("(dc p) q -> p dc q", p=P)
    wq_r = wq_d.rearrange("(dc p) e -> p dc e", p=P)
    wk_r = wk_d.rearrange("(dc p) e -> p dc e", p=P)
    wvh_r = wvh_d.rearrange("(dc p) e -> p dc e", p=P)

    offs = [128 * sum(CAPS[:s]) for s in range(len(CAPS))]  # sc/wt offsets

    def ag(ins_ap, outs_ap):
        nc.gpsimd.collective_compute(
            "AllGather", mybir.AluOpType.bypass, replica_groups=PAIRS,
            ins=[ins_ap], outs=[outs_ap])

    with tile.TileContext(nc) as tc:
        import contextlib
        with contextlib.ExitStack() as ctx:
            cpool = ctx.enter_context(tc.tile_pool(name="cpool", bufs=1))
            qtp = ctx.enter_context(tc.tile_pool(name="qtp", bufs=1))
            ktp = ctx.enter_context(tc.tile_pool(name="ktp", bufs=1))
            vp = ctx.enter_context(tc.tile_pool(name="vp", bufs=1))
            scp = ctx.enter_context(tc.tile_pool(name="scp", bufs=1))
            wtp = ctx.enter_context(tc.tile_pool(name="wtp", bufs=1))
            stp = ctx.enter_context(tc.tile_pool(name="stp", bufs=1))

            # ---- warmup AllGather: absorbs the ~35us firmware wakeup so
            # the real V AllGather starts promptly once its input lands ----
            warm = cpool.tile([P, 16], bf16, name="warm")
            nc.vector.memset(warm[:], 0.0)
            nc.sync.dma_start(warm_i, warm[:])
            ag(warm_i, warm_o)

            QT = qtp.tile([P, DC, 8 * P], bf16, name="QT")
            KT = ktp.tile([P, DC, N], bf16, name="KT")
            V = vp.tile([P, N // P, D], bf16, name="V")

            # ---- projections ----
            with tc.tile_pool(name="wpool", bufs=1) as wpool, \
                 tc.tile_pool(name="xfp", bufs=1) as xfp, \
                 tc.tile_pool(name="xpool", bufs=2) as xpool, \
                 tc.tile_pool(name="cst", bufs=6) as cst, \
                 tc.tile_pool(name="ppj", bufs=6, space="PSUM") as ppj:

                # x^T fully resident (reused by the V then K projections).
                # First 512-col chunk is interleaved with the V weights in
                # dc-quarters so the PE can start after ~0.5MB of DMA; the
                # rest streams behind it, weights after x (nothing below
                # has waits, so the sync queue issues back-to-back).
                xf = xfp.tile([P, DC, N], bf16, name="xf")
                wvh = wpool.tile([P, DC, EH], bf16, name="wvh")
                for q4 in range(4):
                    sl = slice(2 * q4, 2 * q4 + 2)
                    nc.sync.dma_start(xf[:, sl, 0:512], xT_r[:, sl, 0:512])
                    nc.sync.dma_start(wvh[:, sl, :], wvh_r[:, sl, :])
                for c in range(1, 4):
                    nc.sync.dma_start(xf[:, :, c * 512:(c + 1) * 512],
                                      xT_r[:, :, c * 512:(c + 1) * 512])
                wk = wpool.tile([P, DC, D], bf16, name="wk")
                wq = wpool.tile([P, DC, D], bf16, name="wq")
                nc.sync.dma_start(wk[:], wk_r)
                nc.sync.dma_start(wq[:], wq_r)
                mask_sb = cpool.tile([P, 2 * P], f32, name="mask_sb")
                nc.sync.dma_start(mask_sb[:], mask_d)
                ident_f = cpool.tile([P, P], f32, name="ident_f")
                make_identity(nc, ident_f)
                ident = cpool.tile([P, P], bf16, name="ident")
                nc.vector.tensor_copy(ident[:], ident_f[:])

                # V e-half: V[k, 512r + j] for j in 0..511, staged to the
                # AllGather bounce as soon as each 128-row block is cast
                for c in range(4):
                    for kb in range(4):
                        ps = ppj.tile([P, 512], f32, tag="pj", name="ps_v")
                        for dc in range(DC):
                            nc.tensor.matmul(
                                ps,
                                xf[:, dc, (4 * c + kb) * P:(4 * c + kb + 1) * P],
                                wvh[:, dc, :],
                                start=(dc == 0), stop=(dc == DC - 1))
                        vst = cst.tile([P, 512], bf16, tag="cst", name="vst")
                        nc.vector.tensor_copy(vst[:], ps)
                        # scalar (ACT) HWDGE queue: keeps these cast-gated
                        # stores from head-blocking the sync DMA queue
                        nc.scalar.dma_start(vag_i[4 * c + kb], vst[:])
                ag(vag_i, vag_o)

                # K^T in full (duplicated within the pair)
                for c in range(4):
                    for ec in range(DC):
                        ps = ppj.tile([P, 512], f32, tag="pj", name="ps_k")
                        for dc in range(DC):
                            nc.tensor.matmul(
                                ps,
                                wk[:, dc, ec * P:(ec + 1) * P],
                                xf[:, dc, c * 512:(c + 1) * 512],
                                start=(dc == 0), stop=(dc == DC - 1))
                        if ec % 2:  # split copybacks across DVE and ACT
                            nc.vector.tensor_copy(
                                KT[:, ec, c * 512:(c + 1) * 512], ps)
                        else:
                            nc.scalar.activation(
                                KT[:, ec, c * 512:(c + 1) * 512], ps,
                                mybir.ActivationFunctionType.Copy)

                # V readback from the gathered halves (scalar engine queue
                # so other sync DMAs don't queue behind the AG wait);
                # split per half so AV h=0 can start before h=1 lands
                nc.scalar.dma_start(
                    V[:, :, 0:EH], vag_o[0].rearrange("kc p e -> p kc e"))
                nc.scalar.dma_start(
                    V[:, :, EH:D], vag_o[1].rearrange("kc p e -> p kc e"))

                # Q^T full e for own q rows, scaled by 1/32
                for qc in range(2):
                    xs = xpool.tile([P, DC, 512], bf16, tag="xs", name="xs_q")
                    nc.sync.dma_start(
                        xs[:], xqT_r[:, :, qc * 512:(qc + 1) * 512])
                    for ec in range(DC):
                        ps = ppj.tile([P, 512], f32, tag="pj", name="ps_q")
                        for dc in range(DC):
                            nc.tensor.matmul(
                                ps,
                                wq[:, dc, ec * P:(ec + 1) * P],
                                xs[:, dc, :],
                                start=(dc == 0), stop=(dc == DC - 1))
                        nc.vector.tensor_scalar_mul(
                            QT[:, ec, qc * 512:(qc + 1) * 512], ps, 1.0 / 32.0)

            # ---- attention ----
            with tc.tile_pool(name="psc", bufs=3, space="PSUM") as psc, \
                 tc.tile_pool(name="ptr", bufs=2, space="PSUM") as ptr, \
                 tc.tile_pool(name="pav", bufs=3, space="PSUM") as pav, \
                 tc.tile_pool(name="obp", bufs=2) as obp:

                SC = scp.tile([P, SUMCAPS * P], bf16, name="SC")  # exp'd
                WT = wtp.tile([P, SUMCAPS * P], bf16, name="WT")  # transposed
                ST = stp.tile([P, len(CAPS), 6], f32, name="ST")  # rowsums

                # scores + exp + transpose, slot by slot (V-independent)
                for s in range(len(CAPS)):
                    L = CAPS[s] * P
                    off = offs[s]
                    widths = [512] * (L // 512) + ([256] if L % 512 else [])
                    koff = 0
                    for ci, w in enumerate(widths):
                        ps = psc.tile([P, 512], f32, tag="psc", name=f"sc{s}")
                        for ec in range(DC):
                            nc.tensor.matmul(
                                ps[:, :w],
                                QT[:, ec, s * P:(s + 1) * P],
                                KT[:, ec, koff:koff + w],
                                start=(ec == 0), stop=(ec == DC - 1))
                        koff += w
                        if koff == L:  # apply causal mask to last 256 cols
                            nc.vector.tensor_add(
                                ps[:, w - 256:w], ps[:, w - 256:w], mask_sb[:])
                        nc.scalar.activation(
                            SC[:, off + koff - w:off + koff], ps[:, :w],
                            mybir.ActivationFunctionType.Exp,
                            accum_out=ST[:, s, ci:ci + 1])
                    # rowsum = sum of chunk accumulators; reciprocal
                    nc.vector.tensor_reduce(
                        ST[:, s, 4:5], ST[:, s, 0:len(widths)],
                        axis=mybir.AxisListType.X, op=mybir.AluOpType.add)
                    nc.vector.reciprocal(ST[:, s, 5:6], ST[:, s, 4:5])
                    for j in range(CAPS[s]):
                        pt = ptr.tile([P, P], bf16, tag="ptr", name=f"pt{s}")
                        nc.tensor.transpose(
                            pt, SC[:, off + j * P:off + (j + 1) * P], ident)
                        nc.vector.tensor_copy(
                            WT[:, off + j * P:off + (j + 1) * P], pt)

                # AV, slot by slot (first consumer of V); h-outer so the
                # h=0 matmuls only need the first V readback DMA
                for s in range(len(CAPS)):
                    off = offs[s]
                    avs = [pav.tile([P, 512], f32, tag="pav",
                                    name=f"av{s}_{h}") for h in range(2)]
                    for h in range(2):
                        for j in range(CAPS[s]):
                            nc.tensor.matmul(
                                avs[h],
                                WT[:, off + j * P:off + (j + 1) * P],
                                V[:, j, h * 512:(h + 1) * 512],
                                start=(j == 0), stop=(j == CAPS[s] - 1))
                    ob = obp.tile([P, D], f32, tag="ob", name=f"ob{s}")
                    for h in range(2):
                        nc.vector.tensor_scalar_mul(
                            ob[:, h * 512:(h + 1) * 512], avs[h], ST[:, s, 5:6])
                    nc.sync.dma_start(out_d[s * P:(s + 1) * P, :], ob)

    _split_multi_waits(nc)
    return nc


def _host_prep(x, Wq, Wk, Wv):
    """Build per-core input maps."""
    import ml_dtypes
    bf16 = ml_dtypes.bfloat16

    x = np.ascontiguousarray(x, dtype=np.float32)
    tri = np.where(
        np.arange(P)[None, :] <= np.arange(P)[:, None], 0.0, NEG
    ).astype(np.float32)
    mask_even = np.concatenate(  # parity 0: diag block then fully-masked block
        [tri, np.full((P, P), NEG, np.float32)], axis=1)
    mask_odd = np.concatenate(  # parity 1: fully-visible block then diag block
        [np.zeros((P, P), np.float32), tri], axis=1)

    xb = [np.ascontiguousarray(x[bi].T).astype(bf16) for bi in range(B)]
    wq_b = np.ascontiguousarray(Wq, dtype=np.float32).astype(bf16)
    wk_b = np.ascontiguousarray(Wk, dtype=np.float32).astype(bf16)
    wv_b = np.ascontiguousarray(Wv, dtype=np.float32).astype(bf16)

    in_maps = []
    for c in range(NCORES):
        bi, r = c // 2, c % 2
        rbs = [s - 2 + r for s in CAPS]
        xq = np.concatenate([x[bi, rb * P:(rb + 1) * P, :] for rb in rbs],
                            axis=0)
        in_maps.append({
            "xT": xb[bi],
            "xqT": np.ascontiguousarray(xq.T).astype(bf16),
            "wq": wq_b,
            "wk": wk_b,
            "wvh": np.ascontiguousarray(wv_b[:, r * EH:(r + 1) * EH]),
            "mask": mask_odd if r else mask_even,
        })
    return in_maps


def _host_gather(results):
    out = np.empty((B, N, D), dtype=np.float32)
    for c in range(NCORES):
        bi, r = c // 2, c % 2
        res = results[c]["out"]
        for k, s in enumerate(CAPS):
            rb = s - 2 + r
            out[bi, rb * P:(rb + 1) * P, :] = res[k * P:(k + 1) * P, :]
    return out


def kernel(x, Wq, Wk, Wv, _trace=False, _trace_kwargs=None):
    from concourse.bass_utils import run_bass_kernel_spmd

    if "prog" not in _prog_cache:
        _prog_cache["prog"] = _build_program()
    nc = _prog_cache["prog"]

    in_maps = _host_prep(x, Wq, Wk, Wv)
    kw = dict(_trace_kwargs or {})
    res = run_bass_kernel_spmd(nc, in_maps, list(range(NCORES)),
                               trace=_trace, **kw)
    out = _host_gather(res.results)
    if _trace:
        return out, res
    return out
